# revision 20
# baseline (speedup 1.0000x reference)
"""GCN message-passing kernel for 8 trn2 NeuronCores (v2).

Math:  out = segment_sum(h[edge_src], edge_dst) @ W_post + b_post,
       h = data @ W_pre + b_pre.
By linearity:
       out[d] = (sum_{e: dst=d} data[src_e]) @ (W_pre @ W_post)
                + deg[d] * (b_pre @ W_post) + b_post

Sharding: dst-node shards of 12500 per core (fully independent — no
collectives).  Each core gathers bf16 data rows (256 B) for the edges landing
in its shard with dma_gather split across all 4 SWDGE queues (the gather is
Q7-descriptor-emission-bound: queue q runs on GpSimd cores 2q/2q+1, so four
queues quadruple the emission rate), segment-sums them with bf16 one-hot
matmuls on the TensorEngine (PSUM f32 accumulation per 128-node dst block),
applies the host-folded projection W_pre@W_post, and writes its output shard
transposed ([64, shard]); the host re-assembles.

Cell layout: per (dst-block, src-window) cell, slots are padded to a
multiple of 128 (variable chunk counts, ~12% padding vs 25% for fixed).
The program is built from the per-cell chunk counts and cached on them.

Self-contained: numpy + ml_dtypes + concourse imports; shapes hardcoded.
"""

from contextlib import ExitStack

import numpy as np
import ml_dtypes

import concourse.bacc as bacc
import concourse.mybir as mybir
import concourse.tile as tile
from concourse import library_config
from concourse.bass_utils import run_bass_kernel_spmd

F32 = mybir.dt.float32
BF16 = mybir.dt.bfloat16
I16 = mybir.dt.int16
NPBF16 = ml_dtypes.bfloat16


class Cfg:
    N = 100000          # nodes
    DIN = 128           # input features
    DOUT = 64           # output features
    NC = 8              # cores
    SH = 12500          # dst nodes per core
    BS = 128            # dst block size
    NB = 98             # ceil(SH/BS) blocks per core
    NW = 4              # src windows (int16 gather index limit)
    WS = 25000          # window size
    G = 6               # blocks per gather group (6 acc psum banks + 2 out)


def _group_sizes(cfg):
    sizes = []
    b = 0
    while b < cfg.NB:
        sizes.append(min(cfg.G, cfg.NB - b))
        b += cfg.G
    return sizes


def _pack_nodes(v, cfg):
    """Greedy 4-dim bin packing of dst nodes into NC*NB bins of <=BS nodes.

    v: [N, NW] per-node edge counts per src window.  Aims for <=4 chunks
    (512 edges) per (bin, window) cell.  Returns (block_of, pos_of) with
    bin ids in [0, NC*NB).
    """
    n_bins = cfg.NC * cfg.NB
    cap = 4 * cfg.BS
    deg = v.sum(1)
    order = np.argsort(-deg, kind="stable")
    loads = np.zeros((n_bins, cfg.NW), np.int32)
    counts = np.zeros(n_bins, np.int32)
    block_of = np.empty(cfg.N, np.int32)
    pos_of = np.empty(cfg.N, np.int32)
    big = 10 ** 6
    for d in order:
        proj = loads + v[d]
        score = (np.maximum(proj - cap, 0).sum(1)) * big + proj.max(1)
        score[counts >= cfg.BS] = 2 ** 31 - 1
        b = int(np.argmin(score))
        block_of[d] = b
        pos_of[d] = counts[b]
        counts[b] += 1
        loads[b] += v[d]
    return block_of, pos_of, loads


def preprocess(edge_src, edge_dst, cfg=Cfg):
    """Relabel dst nodes into balanced (core, block, pos) slots, then build
    per-core gather-index / dst-local / degree arrays.

    Returns dict with chunks_u [NB, NW], idx_dram, loc_dram, deg_dram, and
    the node->(core, row) maps for host-side output reassembly.
    """
    src = np.asarray(edge_src).astype(np.int64)
    dst = np.asarray(edge_dst).astype(np.int64)

    win = src // cfg.WS
    widx = src - win * cfg.WS

    # --- dst-node relabeling: pack nodes into bins balancing window loads
    v = np.bincount(dst * cfg.NW + win,
                    minlength=cfg.N * cfg.NW).reshape(cfg.N, cfg.NW)
    v = v.astype(np.int32)
    block_of, pos_of, loads = _pack_nodes(v, cfg)

    # group bins of equal chunk signature into cores: bin -> (core, block)
    chunkv = np.maximum(1, -(-loads // cfg.BS))          # [n_bins, NW]
    orderb = np.lexsort(chunkv.T)                        # sort by signature
    core_of_bin = np.empty(cfg.NC * cfg.NB, np.int32)
    blk_of_bin = np.empty(cfg.NC * cfg.NB, np.int32)
    chunks_u = np.empty((cfg.NB, cfg.NW), np.int64)
    for j in range(cfg.NB):
        grp = orderb[cfg.NC * j: cfg.NC * (j + 1)]
        core_of_bin[grp] = np.arange(cfg.NC)
        blk_of_bin[grp] = j
        chunks_u[j] = chunkv[grp].max(0)

    core = core_of_bin[block_of[dst]]
    blk = blk_of_bin[block_of[dst]].astype(np.int64)
    loc = pos_of[dst].astype(np.int64)

    # --- cell slot bases in the group-major layout
    group_sizes = _group_sizes(cfg)
    cell_base = np.zeros((cfg.NB, cfg.NW), np.int64)
    off = 0
    b0 = 0
    for gs in group_sizes:
        for w in range(cfg.NW):
            for bi in range(gs):
                cell_base[b0 + bi, w] = off
                off += chunks_u[b0 + bi, w] * 128
        b0 += gs
    tot_slots = int(off)

    cell = (core * cfg.NB + blk) * cfg.NW + win
    counts_f = np.bincount(cell, minlength=cfg.NC * cfg.NB * cfg.NW)
    order = np.argsort(cell, kind="stable")
    starts = np.zeros(cfg.NC * cfg.NB * cfg.NW, np.int64)
    starts[1:] = np.cumsum(counts_f)[:-1]
    rank = np.arange(len(src)) - starts[cell[order]]

    core_s = core[order]
    slot = cell_base[blk[order], win[order]] + rank

    idx_all = np.zeros((cfg.NC, tot_slots), np.int16)
    loc_all = np.full((cfg.NC, tot_slots), -1.0, np.float32)
    idx_all[core_s, slot] = widx[order].astype(np.int16)
    loc_all[core_s, slot] = loc[order].astype(np.float32)

    # idx wrap: [NC, 16, tot/16] tiled to 128 partitions; per-gather slices
    # are 128-slot aligned so one global wrap works.
    wrapped = idx_all.reshape(cfg.NC, tot_slots // 16, 16).transpose(0, 2, 1)
    idx_dram = np.ascontiguousarray(np.tile(wrapped, (1, 8, 1)))
    loc_dram = np.ascontiguousarray(
        loc_all.reshape(cfg.NC, tot_slots // 128, 128).transpose(0, 2, 1)
    ).astype(NPBF16)

    # node n sits at (core_node[n], row_node[n]) of that core's output
    core_node = core_of_bin[block_of]
    row_node = blk_of_bin[block_of].astype(np.int64) * 128 + pos_of

    deg_dram = np.zeros((cfg.NC, 1, cfg.NB * 128), np.float32)
    degs = np.bincount(dst, minlength=cfg.N).astype(np.float32)
    deg_dram[core_node, 0, row_node] = degs

    return {
        "chunks_u": chunks_u,
        "idx_dram": idx_dram,
        "loc_dram": loc_dram,
        "deg_dram": deg_dram.astype(NPBF16),
        "core_node": core_node,
        "row_node": row_node,
    }


def build_program(chunks_u, cfg=Cfg, reps=1, do_gather="rr", do_onehot=True,
                  do_mm=True):
    """chunks_u: [NB, NW] int array of 128-slot chunk counts per cell."""
    group_sizes = _group_sizes(cfg)
    tot_slots = int(chunks_u.sum()) * 128
    nc = bacc.Bacc("TRN2", target_bir_lowering=False, debug=True,
                   num_swdge_queues=cfg.NW)

    data = nc.dram_tensor("data", [cfg.N, cfg.DIN], BF16, kind="ExternalInput")
    idxs = nc.dram_tensor("idxs", [128, tot_slots // 16], I16,
                          kind="ExternalInput")
    locs = nc.dram_tensor("locs", [128, tot_slots // 128], BF16,
                          kind="ExternalInput")
    deg = nc.dram_tensor("deg", [1, cfg.NB * 128], BF16, kind="ExternalInput")
    iota_in = nc.dram_tensor("iota", [128, 128], BF16, kind="ExternalInput")
    wcomb_in = nc.dram_tensor("wcomb", [cfg.DIN, cfg.DOUT], BF16,
                              kind="ExternalInput")
    bpw_in = nc.dram_tensor("bpw", [1, cfg.DOUT], BF16, kind="ExternalInput")
    bpost_in = nc.dram_tensor("bpost", [1, cfg.DOUT], BF16,
                              kind="ExternalInput")
    out = nc.dram_tensor("out", [cfg.DOUT, cfg.NB * 128], F32,
                         kind="ExternalOutput")

    with tile.TileContext(nc) as tc, ExitStack() as stk:
        nc.gpsimd.load_library(library_config.mlp)
        with (
            tc.tile_pool(name="consts", bufs=1) as cpool,
            tc.tile_pool(name="idxp", bufs=6) as idxp,
            tc.tile_pool(name="locp", bufs=6) as locp,
            tc.tile_pool(name="msgs", bufs=5) as msgsp,
            tc.tile_pool(name="oh", bufs=5) as ohp,
            tc.tile_pool(name="accsb", bufs=3) as accsbp,
            tc.tile_pool(name="outsb", bufs=2) as outsbp,
            tc.tile_pool(name="degp", bufs=2) as degp,
        ):
            iota_sb = cpool.tile([128, 128], BF16)
            wcomb_sb = cpool.tile([cfg.DIN, cfg.DOUT], BF16)
            bpw_sb = cpool.tile([1, cfg.DOUT], BF16)
            bpost_sb = cpool.tile([1, cfg.DOUT], BF16)
            ones_sb = cpool.tile([1, 128], BF16)
            nc.sync.dma_start(out=iota_sb[:], in_=iota_in[:])
            nc.sync.dma_start(out=wcomb_sb[:], in_=wcomb_in[:])
            nc.sync.dma_start(out=bpw_sb[:], in_=bpw_in[:])
            nc.sync.dma_start(out=bpost_sb[:], in_=bpost_in[:])
            nc.vector.memset(ones_sb[:], 1.0)
            m_shared = None
            if not do_gather:
                cmax = 0
                b0t = 0
                for gs in group_sizes:
                    for w in range(cfg.NW):
                        cw = int(chunks_u[b0t:b0t + gs, w].sum())
                        cmax = max(cmax, cw)
                    b0t += gs
                m_shared = cpool.tile([128, cmax, cfg.DIN], BF16)
                nc.vector.memset(m_shared[:], 0.0)

            psacc = stk.enter_context(
                tc.tile_pool(name="psacc", bufs=cfg.G, space="PSUM"))
            psout = stk.enter_context(
                tc.tile_pool(name="psout", bufs=2, space="PSUM"))
            for _rep in range(reps):
                off = 0      # slot offset
                b0 = 0       # first block of group
                for gi, gs in enumerate(group_sizes):
                    deg_t = degp.tile([1, gs * 128], BF16)
                    nc.sync.dma_start(out=deg_t[:],
                                      in_=deg[:, b0 * 128: (b0 + gs) * 128])
                    accs = ([psacc.tile([128, 128], F32,
                                        name=f"acc{_rep}_{b0}_{i}",
                                        tag="acc")[:]
                             for i in range(gs)] if do_mm else [])
                    # per block: windows in which it has its first/last chunk
                    nch = [[int(chunks_u[b0 + bi, w]) for w in range(cfg.NW)]
                           for bi in range(gs)]
                    for w in range(cfg.NW):
                        Cw = sum(nch[bi][w] for bi in range(gs))
                        n = Cw * 128
                        idx_t = idxp.tile([128, n // 16], I16)
                        nc.sync.dma_start(
                            out=idx_t[:],
                            in_=idxs[:, off // 16: (off + n) // 16])
                        loc_t = locp.tile([128, Cw], BF16)
                        nc.sync.dma_start(
                            out=loc_t[:],
                            in_=locs[:, off // 128: (off + n) // 128])
                        if do_gather == "rr":
                            m_t = msgsp.tile([128, Cw, cfg.DIN], BF16)
                            nc.gpsimd.dma_gather(
                                m_t[:], data[w * cfg.WS: (w + 1) * cfg.WS, :],
                                idx_t[:], n, n, cfg.DIN, single_packet=False,
                                queue_num=w)
                        elif do_gather:
                            m_t = msgsp.tile([128, Cw, cfg.DIN], BF16)
                            c0 = 0
                            for q in range(cfg.NW):
                                cq = Cw // cfg.NW + (1 if q < Cw % cfg.NW
                                                     else 0)
                                if cq == 0:
                                    continue
                                nq = cq * 128
                                nc.gpsimd.dma_gather(
                                    m_t[:, c0:c0 + cq, :],
                                    data[w * cfg.WS: (w + 1) * cfg.WS, :],
                                    idx_t[:, c0 * 8:(c0 + cq) * 8],
                                    nq, nq, cfg.DIN, single_packet=False,
                                    queue_num=q)
                                c0 += cq
                        else:
                            m_t = m_shared
                        o_t = ohp.tile([128, Cw, 128], BF16)
                        if do_onehot:
                            nc.vector.tensor_tensor(
                                out=o_t[:],
                                in0=loc_t[:].unsqueeze(2).broadcast_to(
                                    [128, Cw, 128]),
                                in1=iota_sb[:].unsqueeze(1).broadcast_to(
                                    [128, Cw, 128]),
                                op=mybir.AluOpType.is_equal)
                        ch = 0
                        if do_mm:
                            for bi in range(gs):
                                for cu in range(nch[bi][w]):
                                    nc.tensor.matmul(
                                        out=accs[bi],
                                        lhsT=m_t[:, ch, :],
                                        rhs=o_t[:, ch, :],
                                        start=(w == 0 and cu == 0),
                                        stop=(w == cfg.NW - 1
                                              and cu == nch[bi][w] - 1))
                                    ch += 1
                        off += n

                    out_t = outsbp.tile([cfg.DOUT, gs * 128], F32)
                    if do_mm:
                        for bi in range(gs):
                            acc_sb = accsbp.tile([128, 128], BF16)
                            nc.scalar.copy(acc_sb[:], accs[bi])
                            outp = psout.tile([cfg.DOUT, 128], F32)
                            nc.tensor.matmul(out=outp[:], lhsT=wcomb_sb[:],
                                             rhs=acc_sb[:], start=True,
                                             stop=False)
                            nc.tensor.matmul(out=outp[:], lhsT=bpw_sb[:],
                                             rhs=deg_t[:,
                                                       bi * 128:(bi + 1) * 128],
                                             start=False, stop=False)
                            nc.tensor.matmul(out=outp[:], lhsT=bpost_sb[:],
                                             rhs=ones_sb[:], start=False,
                                             stop=True)
                            nc.scalar.copy(out_t[:, bi * 128:(bi + 1) * 128],
                                           outp[:])
                    else:
                        nc.vector.memset(out_t[:], 0.0)
                    nc.sync.dma_start(
                        out=out[:, b0 * 128:(b0 + gs) * 128], in_=out_t[:])
                    b0 += gs
    nc.compile()
    return nc


_PROGRAM_CACHE = {}


def _get_program(chunks_u, cfg=Cfg, reps=1):
    key = (chunks_u.tobytes(), reps)
    if key not in _PROGRAM_CACHE:
        _PROGRAM_CACHE[key] = build_program(chunks_u, cfg, reps)
    return _PROGRAM_CACHE[key]


_PRE_CACHE = {}


def _preprocess_cached(edge_src, edge_dst, cfg=Cfg):
    import hashlib
    key = hashlib.sha1(np.asarray(edge_src).tobytes()
                       + np.asarray(edge_dst).tobytes()).hexdigest()
    if key not in _PRE_CACHE:
        _PRE_CACHE[key] = preprocess(edge_src, edge_dst, cfg)
    return _PRE_CACHE[key]


def make_in_maps(data, edge_src, edge_dst, W_pre, b_pre, W_post, b_post,
                 cfg=Cfg):
    pre = _preprocess_cached(edge_src, edge_dst, cfg)
    data16 = np.ascontiguousarray(
        np.asarray(data, dtype=np.float32)).astype(NPBF16)
    iota = np.tile(np.arange(128, dtype=np.float32),
                   (128, 1)).astype(NPBF16)
    wp = np.asarray(W_pre, np.float64)
    wq = np.asarray(W_post, np.float64)
    wcomb = (wp @ wq).astype(np.float32).astype(NPBF16)
    bpw = (np.asarray(b_pre, np.float64) @ wq).reshape(1, cfg.DOUT)
    bpw = bpw.astype(np.float32).astype(NPBF16)
    bpost = np.asarray(b_post, np.float32).reshape(1, cfg.DOUT).astype(NPBF16)
    in_maps = []
    for c in range(cfg.NC):
        in_maps.append({
            "data": data16,
            "idxs": pre["idx_dram"][c],
            "locs": pre["loc_dram"][c],
            "deg": pre["deg_dram"][c],
            "iota": iota,
            "wcomb": wcomb,
            "bpw": bpw,
            "bpost": bpost,
        })
    return pre, in_maps


def assemble_output(pre, core_outs, cfg=Cfg):
    """core_outs: list of [DOUT, NB*128] arrays -> full [N, DOUT]."""
    out = np.empty((cfg.N, cfg.DOUT), np.float32)
    core_node = pre["core_node"]
    row_node = pre["row_node"]
    for c in range(cfg.NC):
        mask = core_node == c
        out[mask] = core_outs[c][:, row_node[mask]].T
    return out


def kernel(data, edge_src, edge_dst, W_pre, b_pre, W_post, b_post):
    cfg = Cfg
    pre, in_maps = make_in_maps(data, edge_src, edge_dst, W_pre, b_pre,
                                W_post, b_post, cfg)
    nc = _get_program(pre["chunks_u"], cfg)
    res = run_bass_kernel_spmd(nc, in_maps, list(range(cfg.NC)), trace=False)
    return assemble_output(pre, [res.results[c]["out"]
                                 for c in range(cfg.NC)], cfg)


# revision 31
# speedup vs baseline: 1.1483x; 1.1483x over previous
"""GCN message-passing kernel for 8 trn2 NeuronCores (v2).

Math:  out = segment_sum(h[edge_src], edge_dst) @ W_post + b_post,
       h = data @ W_pre + b_pre.
By linearity:
       out[d] = (sum_{e: dst=d} data[src_e]) @ (W_pre @ W_post)
                + deg[d] * (b_pre @ W_post) + b_post

Sharding: dst-node shards of ~12500 per core (fully independent — no
collectives).  Each core gathers bf16 data rows (256 B) for the edges
landing in its shard with dma_gather on all 4 SWDGE queues (the gather is
Q7-descriptor-emission-bound: queue q runs on GpSimd cores 2q/2q+1, so the
(group, window) gathers run round-robin queue=window, quadrupling emission
rate), segment-sums them with bf16 one-hot matmuls on the TensorEngine
(PSUM f32 accumulation per 128-node dst block), applies the host-folded
projection W_pre@W_post, and writes its output shard transposed
([64, shard]); the host re-assembles via the node permutation.

Node relabeling: dst nodes are greedily bin-packed into (core, block)
bins of <=128 nodes balancing the 4 src-window loads, so nearly every
(block, window) cell is exactly 4 chunks (512 slots) — ~0.8% gather
padding instead of ~25% for the unpacked layout.  Bins with equal chunk
signatures are grouped across cores so one shared SPMD program fits all
8 cores.  The program is built from the per-cell chunk counts and cached.

Self-contained: numpy + ml_dtypes + concourse imports; shapes hardcoded.
"""

from contextlib import ExitStack

import numpy as np
import ml_dtypes

import concourse.bacc as bacc
import concourse.mybir as mybir
import concourse.tile as tile
from concourse import library_config
from concourse.bass_utils import run_bass_kernel_spmd

F32 = mybir.dt.float32
BF16 = mybir.dt.bfloat16
I16 = mybir.dt.int16
NPBF16 = ml_dtypes.bfloat16


class Cfg:
    N = 100000          # nodes
    DIN = 128           # input features
    DOUT = 64           # output features
    NC = 8              # cores
    SH = 12500          # dst nodes per core
    BS = 128            # dst block size
    NB = 98             # ceil(SH/BS) blocks per core
    NW = 4              # src windows (int16 gather index limit)
    WS = 25000          # window size
    G = 6               # blocks per gather group (6 acc psum banks + 2 out)


def _group_sizes(cfg):
    sizes = []
    b = 0
    while b < cfg.NB:
        sizes.append(min(cfg.G, cfg.NB - b))
        b += cfg.G
    return sizes


def _pack_nodes(v, cfg):
    """Greedy 4-dim bin packing of dst nodes into NC*NB bins of <=BS nodes.

    v: [N, NW] per-node edge counts per src window.  Aims for <=4 chunks
    (512 edges) per (bin, window) cell.  Returns (block_of, pos_of) with
    bin ids in [0, NC*NB).
    """
    n_bins = cfg.NC * cfg.NB
    cap = 4 * cfg.BS
    deg = v.sum(1)
    order = np.argsort(-deg, kind="stable")
    loads = np.zeros((n_bins, cfg.NW), np.int32)
    counts = np.zeros(n_bins, np.int32)
    block_of = np.empty(cfg.N, np.int32)
    pos_of = np.empty(cfg.N, np.int32)
    big = 10 ** 6
    for d in order:
        proj = loads + v[d]
        score = (np.maximum(proj - cap, 0).sum(1)) * big + proj.max(1)
        score[counts >= cfg.BS] = 2 ** 31 - 1
        b = int(np.argmin(score))
        block_of[d] = b
        pos_of[d] = counts[b]
        counts[b] += 1
        loads[b] += v[d]
    return block_of, pos_of, loads


def preprocess(edge_src, edge_dst, cfg=Cfg):
    """Relabel dst nodes into balanced (core, block, pos) slots, then build
    per-core gather-index / dst-local / degree arrays.

    Returns dict with chunks_u [NB, NW], idx_dram, loc_dram, deg_dram, and
    the node->(core, row) maps for host-side output reassembly.
    """
    src = np.asarray(edge_src).astype(np.int64)
    dst = np.asarray(edge_dst).astype(np.int64)

    win = src // cfg.WS
    widx = src - win * cfg.WS

    # --- dst-node relabeling: pack nodes into bins balancing window loads
    v = np.bincount(dst * cfg.NW + win,
                    minlength=cfg.N * cfg.NW).reshape(cfg.N, cfg.NW)
    v = v.astype(np.int32)
    block_of, pos_of, loads = _pack_nodes(v, cfg)

    # group bins of equal chunk signature into cores: bin -> (core, block)
    chunkv = np.maximum(1, -(-loads // cfg.BS))          # [n_bins, NW]
    orderb = np.lexsort(chunkv.T)                        # sort by signature
    core_of_bin = np.empty(cfg.NC * cfg.NB, np.int32)
    blk_of_bin = np.empty(cfg.NC * cfg.NB, np.int32)
    chunks_u = np.empty((cfg.NB, cfg.NW), np.int64)
    for j in range(cfg.NB):
        grp = orderb[cfg.NC * j: cfg.NC * (j + 1)]
        core_of_bin[grp] = np.arange(cfg.NC)
        blk_of_bin[grp] = j
        chunks_u[j] = chunkv[grp].max(0)

    core = core_of_bin[block_of[dst]]
    blk = blk_of_bin[block_of[dst]].astype(np.int64)
    loc = pos_of[dst].astype(np.int64)

    # --- cell slot bases in the group-major layout
    group_sizes = _group_sizes(cfg)
    cell_base = np.zeros((cfg.NB, cfg.NW), np.int64)
    off = 0
    b0 = 0
    for gs in group_sizes:
        for w in range(cfg.NW):
            for bi in range(gs):
                cell_base[b0 + bi, w] = off
                off += chunks_u[b0 + bi, w] * 128
        b0 += gs
    tot_slots = int(off)

    cell = (core * cfg.NB + blk) * cfg.NW + win
    counts_f = np.bincount(cell, minlength=cfg.NC * cfg.NB * cfg.NW)
    order = np.argsort(cell, kind="stable")
    starts = np.zeros(cfg.NC * cfg.NB * cfg.NW, np.int64)
    starts[1:] = np.cumsum(counts_f)[:-1]
    rank = np.arange(len(src)) - starts[cell[order]]

    core_s = core[order]
    slot = cell_base[blk[order], win[order]] + rank

    idx_all = np.zeros((cfg.NC, tot_slots), np.int16)
    loc_all = np.full((cfg.NC, tot_slots), -1.0, np.float32)
    idx_all[core_s, slot] = widx[order].astype(np.int16)
    loc_all[core_s, slot] = loc[order].astype(np.float32)

    # idx wrap: [NC, 16, tot/16] tiled to 128 partitions; per-gather slices
    # are 128-slot aligned so one global wrap works.
    wrapped = idx_all.reshape(cfg.NC, tot_slots // 16, 16).transpose(0, 2, 1)
    idx_dram = np.ascontiguousarray(np.tile(wrapped, (1, 8, 1)))
    loc_dram = np.ascontiguousarray(
        loc_all.reshape(cfg.NC, tot_slots // 128, 128).transpose(0, 2, 1)
    ).astype(NPBF16)

    # node n sits at (core_node[n], row_node[n]) of that core's output
    core_node = core_of_bin[block_of]
    row_node = blk_of_bin[block_of].astype(np.int64) * 128 + pos_of

    deg_dram = np.zeros((cfg.NC, 1, cfg.NB * 128), np.float32)
    degs = np.bincount(dst, minlength=cfg.N).astype(np.float32)
    deg_dram[core_node, 0, row_node] = degs

    return {
        "chunks_u": chunks_u,
        "idx_dram": idx_dram,
        "loc_dram": loc_dram,
        "deg_dram": deg_dram.astype(NPBF16),
        "core_node": core_node,
        "row_node": row_node,
    }


def build_program(chunks_u, cfg=Cfg, reps=1, do_gather="rr", do_onehot=True,
                  do_mm=True):
    """chunks_u: [NB, NW] int array of 128-slot chunk counts per cell."""
    group_sizes = _group_sizes(cfg)
    tot_slots = int(chunks_u.sum()) * 128
    nc = bacc.Bacc("TRN2", target_bir_lowering=False, debug=True,
                   num_swdge_queues=cfg.NW)

    data = nc.dram_tensor("data", [cfg.N, cfg.DIN], BF16, kind="ExternalInput")
    idxs = nc.dram_tensor("idxs", [128, tot_slots // 16], I16,
                          kind="ExternalInput")
    locs = nc.dram_tensor("locs", [128, tot_slots // 128], BF16,
                          kind="ExternalInput")
    deg = nc.dram_tensor("deg", [1, cfg.NB * 128], BF16, kind="ExternalInput")
    iota_in = nc.dram_tensor("iota", [128, 128], BF16, kind="ExternalInput")
    wcomb_in = nc.dram_tensor("wcomb", [cfg.DIN, cfg.DOUT], BF16,
                              kind="ExternalInput")
    bpw_in = nc.dram_tensor("bpw", [1, cfg.DOUT], BF16, kind="ExternalInput")
    bpost_in = nc.dram_tensor("bpost", [1, cfg.DOUT], BF16,
                              kind="ExternalInput")
    out = nc.dram_tensor("out", [cfg.DOUT, cfg.NB * 128], F32,
                         kind="ExternalOutput")

    with tile.TileContext(nc) as tc, ExitStack() as stk:
        nc.gpsimd.load_library(library_config.mlp)
        with (
            tc.tile_pool(name="consts", bufs=1) as cpool,
            tc.tile_pool(name="idxp", bufs=6) as idxp,
            tc.tile_pool(name="locp", bufs=6) as locp,
            tc.tile_pool(name="msgs", bufs=6) as msgsp,
            tc.tile_pool(name="oh", bufs=(3 if do_onehot == "group"
                                          else 5)) as ohp,
            tc.tile_pool(name="accsb", bufs=3) as accsbp,
            tc.tile_pool(name="outsb", bufs=2) as outsbp,
            tc.tile_pool(name="degp", bufs=2) as degp,
        ):
            iota_sb = cpool.tile([128, 128], BF16)
            wcomb_sb = cpool.tile([cfg.DIN, cfg.DOUT], BF16)
            bpw_sb = cpool.tile([1, cfg.DOUT], BF16)
            bpost_sb = cpool.tile([1, cfg.DOUT], BF16)
            ones_sb = cpool.tile([1, 128], BF16)
            nc.sync.dma_start(out=iota_sb[:], in_=iota_in[:])
            nc.sync.dma_start(out=wcomb_sb[:], in_=wcomb_in[:])
            nc.sync.dma_start(out=bpw_sb[:], in_=bpw_in[:])
            nc.sync.dma_start(out=bpost_sb[:], in_=bpost_in[:])
            nc.vector.memset(ones_sb[:], 1.0)
            m_shared = o_shared = None
            if not do_gather or not do_onehot:
                cmax = 0
                b0t = 0
                for gs in group_sizes:
                    for w in range(cfg.NW):
                        cw = int(chunks_u[b0t:b0t + gs, w].sum())
                        cmax = max(cmax, cw)
                    b0t += gs
                if not do_gather:
                    m_shared = cpool.tile([128, cmax, cfg.DIN], BF16)
                    nc.vector.memset(m_shared[:], 0.0)
                if not do_onehot:
                    o_shared = cpool.tile([128, cmax, 128], BF16)
                    nc.vector.memset(o_shared[:], 0.0)

            psacc = stk.enter_context(
                tc.tile_pool(name="psacc", bufs=cfg.G, space="PSUM"))
            psout = stk.enter_context(
                tc.tile_pool(name="psout", bufs=2, space="PSUM"))
            for _rep in range(reps):
                off = 0      # slot offset
                b0 = 0       # first block of group
                for gi, gs in enumerate(group_sizes):
                    deg_t = degp.tile([1, gs * 128], BF16)
                    nc.sync.dma_start(out=deg_t[:],
                                      in_=deg[:, b0 * 128: (b0 + gs) * 128])
                    accs = ([psacc.tile([128, 128], F32,
                                        name=f"acc{_rep}_{b0}_{i}",
                                        tag="acc")[:]
                             for i in range(gs)] if do_mm else [])
                    # per block: windows in which it has its first/last chunk
                    nch = [[int(chunks_u[b0 + bi, w]) for w in range(cfg.NW)]
                           for bi in range(gs)]
                    o_group = None
                    if do_onehot == "group":
                        Cg = int(chunks_u[b0:b0 + gs, :].sum())
                        locg_t = locp.tile([128, Cg], BF16)
                        nc.sync.dma_start(
                            out=locg_t[:],
                            in_=locs[:, off // 128: off // 128 + Cg])
                        o_group = ohp.tile([128, Cg, 128], BF16)
                        nc.vector.tensor_tensor(
                            out=o_group[:],
                            in0=locg_t[:].unsqueeze(2).broadcast_to(
                                [128, Cg, 128]),
                            in1=iota_sb[:].unsqueeze(1).broadcast_to(
                                [128, Cg, 128]),
                            op=mybir.AluOpType.is_equal)
                        cg0 = 0
                    for w in range(cfg.NW):
                        Cw = sum(nch[bi][w] for bi in range(gs))
                        n = Cw * 128
                        idx_t = idxp.tile([128, n // 16], I16)
                        nc.sync.dma_start(
                            out=idx_t[:],
                            in_=idxs[:, off // 16: (off + n) // 16])
                        if do_onehot is True:
                            loc_t = locp.tile([128, Cw], BF16)
                            nc.sync.dma_start(
                                out=loc_t[:],
                                in_=locs[:, off // 128: (off + n) // 128])
                        if do_gather == "rr":
                            m_t = msgsp.tile([128, Cw, cfg.DIN], BF16)
                            nc.gpsimd.dma_gather(
                                m_t[:], data[w * cfg.WS: (w + 1) * cfg.WS, :],
                                idx_t[:], n, n, cfg.DIN, single_packet=False,
                                queue_num=w)
                        elif do_gather:
                            m_t = msgsp.tile([128, Cw, cfg.DIN], BF16)
                            c0 = 0
                            for q in range(cfg.NW):
                                cq = Cw // cfg.NW + (1 if q < Cw % cfg.NW
                                                     else 0)
                                if cq == 0:
                                    continue
                                nq = cq * 128
                                nc.gpsimd.dma_gather(
                                    m_t[:, c0:c0 + cq, :],
                                    data[w * cfg.WS: (w + 1) * cfg.WS, :],
                                    idx_t[:, c0 * 8:(c0 + cq) * 8],
                                    nq, nq, cfg.DIN, single_packet=False,
                                    queue_num=q)
                                c0 += cq
                        else:
                            m_t = m_shared
                        if do_onehot == "group":
                            o_t = None
                        elif do_onehot:
                            o_t = ohp.tile([128, Cw, 128], BF16)
                            nc.vector.tensor_tensor(
                                out=o_t[:],
                                in0=loc_t[:].unsqueeze(2).broadcast_to(
                                    [128, Cw, 128]),
                                in1=iota_sb[:].unsqueeze(1).broadcast_to(
                                    [128, Cw, 128]),
                                op=mybir.AluOpType.is_equal)
                        else:
                            o_t = o_shared
                        ch = 0
                        if do_mm:
                            for bi in range(gs):
                                for cu in range(nch[bi][w]):
                                    rhs_ap = (o_group[:, cg0 + ch, :]
                                              if do_onehot == "group"
                                              else o_t[:, ch, :])
                                    nc.tensor.matmul(
                                        out=accs[bi],
                                        lhsT=m_t[:, ch, :],
                                        rhs=rhs_ap,
                                        start=(w == 0 and cu == 0),
                                        stop=(w == cfg.NW - 1
                                              and cu == nch[bi][w] - 1))
                                    ch += 1
                        if do_onehot == "group":
                            cg0 += Cw
                        off += n

                    out_t = outsbp.tile([cfg.DOUT, gs * 128], F32)
                    if do_mm:
                        for bi in range(gs):
                            acc_sb = accsbp.tile([128, 128], BF16)
                            nc.scalar.copy(acc_sb[:], accs[bi])
                            outp = psout.tile([cfg.DOUT, 128], F32)
                            nc.tensor.matmul(out=outp[:], lhsT=wcomb_sb[:],
                                             rhs=acc_sb[:], start=True,
                                             stop=False)
                            nc.tensor.matmul(out=outp[:], lhsT=bpw_sb[:],
                                             rhs=deg_t[:,
                                                       bi * 128:(bi + 1) * 128],
                                             start=False, stop=False)
                            nc.tensor.matmul(out=outp[:], lhsT=bpost_sb[:],
                                             rhs=ones_sb[:], start=False,
                                             stop=True)
                            nc.scalar.copy(out_t[:, bi * 128:(bi + 1) * 128],
                                           outp[:])
                    else:
                        nc.vector.memset(out_t[:], 0.0)
                    nc.sync.dma_start(
                        out=out[:, b0 * 128:(b0 + gs) * 128], in_=out_t[:])
                    b0 += gs
    nc.compile()
    return nc


_PROGRAM_CACHE = {}


def _get_program(chunks_u, cfg=Cfg, reps=1):
    key = (chunks_u.tobytes(), reps)
    if key not in _PROGRAM_CACHE:
        _PROGRAM_CACHE[key] = build_program(chunks_u, cfg, reps)
    return _PROGRAM_CACHE[key]


_PRE_CACHE = {}


def _preprocess_cached(edge_src, edge_dst, cfg=Cfg):
    import hashlib
    key = hashlib.sha1(np.asarray(edge_src).tobytes()
                       + np.asarray(edge_dst).tobytes()).hexdigest()
    if key not in _PRE_CACHE:
        _PRE_CACHE[key] = preprocess(edge_src, edge_dst, cfg)
    return _PRE_CACHE[key]


def make_in_maps(data, edge_src, edge_dst, W_pre, b_pre, W_post, b_post,
                 cfg=Cfg):
    pre = _preprocess_cached(edge_src, edge_dst, cfg)
    data16 = np.ascontiguousarray(
        np.asarray(data, dtype=np.float32)).astype(NPBF16)
    iota = np.tile(np.arange(128, dtype=np.float32),
                   (128, 1)).astype(NPBF16)
    wp = np.asarray(W_pre, np.float64)
    wq = np.asarray(W_post, np.float64)
    wcomb = (wp @ wq).astype(np.float32).astype(NPBF16)
    bpw = (np.asarray(b_pre, np.float64) @ wq).reshape(1, cfg.DOUT)
    bpw = bpw.astype(np.float32).astype(NPBF16)
    bpost = np.asarray(b_post, np.float32).reshape(1, cfg.DOUT).astype(NPBF16)
    in_maps = []
    for c in range(cfg.NC):
        in_maps.append({
            "data": data16,
            "idxs": pre["idx_dram"][c],
            "locs": pre["loc_dram"][c],
            "deg": pre["deg_dram"][c],
            "iota": iota,
            "wcomb": wcomb,
            "bpw": bpw,
            "bpost": bpost,
        })
    return pre, in_maps


def assemble_output(pre, core_outs, cfg=Cfg):
    """core_outs: list of [DOUT, NB*128] arrays -> full [N, DOUT]."""
    out = np.empty((cfg.N, cfg.DOUT), np.float32)
    core_node = pre["core_node"]
    row_node = pre["row_node"]
    for c in range(cfg.NC):
        mask = core_node == c
        out[mask] = core_outs[c][:, row_node[mask]].T
    return out


def kernel(data, edge_src, edge_dst, W_pre, b_pre, W_post, b_post):
    cfg = Cfg
    pre, in_maps = make_in_maps(data, edge_src, edge_dst, W_pre, b_pre,
                                W_post, b_post, cfg)
    nc = _get_program(pre["chunks_u"], cfg)
    try:
        res = run_bass_kernel_spmd(nc, in_maps, list(range(cfg.NC)),
                                   trace=False)
    except ModuleNotFoundError:
        # BASS_TRACE set but the axon NTFF hook isn't importable here —
        # retry with tracing force-disabled rather than failing the run.
        import os
        os.environ["BASS_NEVER_TRACE"] = "1"
        res = run_bass_kernel_spmd(nc, in_maps, list(range(cfg.NC)),
                                   trace=False)
    return assemble_output(pre, [res.results[c]["out"]
                                 for c in range(cfg.NC)], cfg)


# revision 40
# speedup vs baseline: 1.1544x; 1.0053x over previous
"""GCN message-passing kernel for 8 trn2 NeuronCores (v2).

Math:  out = segment_sum(h[edge_src], edge_dst) @ W_post + b_post,
       h = data @ W_pre + b_pre.
By linearity:
       out[d] = (sum_{e: dst=d} data[src_e]) @ (W_pre @ W_post)
                + deg[d] * (b_pre @ W_post) + b_post

Sharding: dst-node shards of ~12500 per core (fully independent — no
collectives).  Each core gathers bf16 data rows (256 B) for the edges
landing in its shard with dma_gather on all 4 SWDGE queues (the gather is
Q7-descriptor-emission-bound: queue q runs on GpSimd cores 2q/2q+1, so the
(group, window) gathers run round-robin queue=window, quadrupling emission
rate), segment-sums them with bf16 one-hot matmuls on the TensorEngine
(PSUM f32 accumulation per 128-node dst block), applies the host-folded
projection W_pre@W_post, and writes its output shard transposed
([64, shard]); the host re-assembles via the node permutation.

Node relabeling: dst nodes are greedily bin-packed into (core, block)
bins of <=128 nodes balancing the 4 src-window loads, so nearly every
(block, window) cell is exactly 4 chunks (512 slots) — ~0.8% gather
padding instead of ~25% for the unpacked layout.  Bins with equal chunk
signatures are grouped across cores so one shared SPMD program fits all
8 cores.  The program is built from the per-cell chunk counts and cached.

Self-contained: numpy + ml_dtypes + concourse imports; shapes hardcoded.
"""

from contextlib import ExitStack

import numpy as np
import ml_dtypes

import concourse.bacc as bacc
import concourse.mybir as mybir
import concourse.tile as tile
from concourse import library_config
from concourse.bass_utils import run_bass_kernel_spmd

F32 = mybir.dt.float32
BF16 = mybir.dt.bfloat16
I16 = mybir.dt.int16
NPBF16 = ml_dtypes.bfloat16


class Cfg:
    N = 100000          # nodes
    DIN = 128           # input features
    DOUT = 64           # output features
    NC = 8              # cores
    SH = 12500          # dst nodes per core
    BS = 128            # dst block size
    NB = 98             # ceil(SH/BS) blocks per core
    NW = 4              # src windows (int16 gather index limit)
    WS = 25000          # window size
    G = 6               # blocks per gather group (6 acc psum banks + 2 out)


def _group_sizes(cfg):
    sizes = []
    b = 0
    while b < cfg.NB:
        sizes.append(min(cfg.G, cfg.NB - b))
        b += cfg.G
    return sizes


def _pack_nodes(v, cfg):
    """Greedy 4-dim bin packing of dst nodes into NC*NB bins of <=BS nodes.

    v: [N, NW] per-node edge counts per src window.  Aims for <=4 chunks
    (512 edges) per (bin, window) cell.  Returns (block_of, pos_of) with
    bin ids in [0, NC*NB).
    """
    n_bins = cfg.NC * cfg.NB
    cap = 4 * cfg.BS
    deg = v.sum(1)
    order = np.argsort(-deg, kind="stable")
    loads = np.zeros((n_bins, cfg.NW), np.int32)
    counts = np.zeros(n_bins, np.int32)
    block_of = np.empty(cfg.N, np.int32)
    pos_of = np.empty(cfg.N, np.int32)
    big = 10 ** 6
    for d in order:
        proj = loads + v[d]
        score = (np.maximum(proj - cap, 0).sum(1)) * big + proj.max(1)
        score[counts >= cfg.BS] = 2 ** 31 - 1
        b = int(np.argmin(score))
        block_of[d] = b
        pos_of[d] = counts[b]
        counts[b] += 1
        loads[b] += v[d]
    return block_of, pos_of, loads


def preprocess(edge_src, edge_dst, cfg=Cfg):
    """Relabel dst nodes into balanced (core, block, pos) slots, then build
    per-core gather-index / dst-local / degree arrays.

    Returns dict with chunks_u [NB, NW], idx_dram, loc_dram, deg_dram, and
    the node->(core, row) maps for host-side output reassembly.
    """
    src = np.asarray(edge_src).astype(np.int64)
    dst = np.asarray(edge_dst).astype(np.int64)

    win = src // cfg.WS
    widx = src - win * cfg.WS

    # --- dst-node relabeling: pack nodes into bins balancing window loads
    v = np.bincount(dst * cfg.NW + win,
                    minlength=cfg.N * cfg.NW).reshape(cfg.N, cfg.NW)
    v = v.astype(np.int32)
    block_of, pos_of, loads = _pack_nodes(v, cfg)

    # group bins of equal chunk signature into cores: bin -> (core, block)
    chunkv = np.maximum(1, -(-loads // cfg.BS))          # [n_bins, NW]
    orderb = np.lexsort(chunkv.T)                        # sort by signature
    core_of_bin = np.empty(cfg.NC * cfg.NB, np.int32)
    blk_of_bin = np.empty(cfg.NC * cfg.NB, np.int32)
    chunks_u = np.empty((cfg.NB, cfg.NW), np.int64)
    for j in range(cfg.NB):
        grp = orderb[cfg.NC * j: cfg.NC * (j + 1)]
        core_of_bin[grp] = np.arange(cfg.NC)
        blk_of_bin[grp] = j
        chunks_u[j] = chunkv[grp].max(0)

    core = core_of_bin[block_of[dst]]
    blk = blk_of_bin[block_of[dst]].astype(np.int64)
    loc = pos_of[dst].astype(np.int64)

    # --- cell slot bases in the group-major layout
    group_sizes = _group_sizes(cfg)
    cell_base = np.zeros((cfg.NB, cfg.NW), np.int64)
    off = 0
    b0 = 0
    for gs in group_sizes:
        for w in range(cfg.NW):
            for bi in range(gs):
                cell_base[b0 + bi, w] = off
                off += chunks_u[b0 + bi, w] * 128
        b0 += gs
    tot_slots = int(off)

    cell = (core * cfg.NB + blk) * cfg.NW + win
    counts_f = np.bincount(cell, minlength=cfg.NC * cfg.NB * cfg.NW)
    order = np.argsort(cell, kind="stable")
    starts = np.zeros(cfg.NC * cfg.NB * cfg.NW, np.int64)
    starts[1:] = np.cumsum(counts_f)[:-1]
    rank = np.arange(len(src)) - starts[cell[order]]

    core_s = core[order]
    slot = cell_base[blk[order], win[order]] + rank

    idx_all = np.zeros((cfg.NC, tot_slots), np.int16)
    loc_all = np.full((cfg.NC, tot_slots), -1.0, np.float32)
    idx_all[core_s, slot] = widx[order].astype(np.int16)
    loc_all[core_s, slot] = loc[order].astype(np.float32)

    # idx wrap: [NC, 16, tot/16] tiled to 128 partitions; per-gather slices
    # are 128-slot aligned so one global wrap works.
    wrapped = idx_all.reshape(cfg.NC, tot_slots // 16, 16).transpose(0, 2, 1)
    idx_dram = np.ascontiguousarray(np.tile(wrapped, (1, 8, 1)))
    loc_dram = np.ascontiguousarray(
        loc_all.reshape(cfg.NC, tot_slots // 128, 128).transpose(0, 2, 1)
    ).astype(NPBF16)

    # node n sits at (core_node[n], row_node[n]) of that core's output
    core_node = core_of_bin[block_of]
    row_node = blk_of_bin[block_of].astype(np.int64) * 128 + pos_of

    deg_dram = np.zeros((cfg.NC, 1, cfg.NB * 128), np.float32)
    degs = np.bincount(dst, minlength=cfg.N).astype(np.float32)
    deg_dram[core_node, 0, row_node] = degs

    return {
        "chunks_u": chunks_u,
        "idx_dram": idx_dram,
        "loc_dram": loc_dram,
        "deg_dram": deg_dram.astype(NPBF16),
        "core_node": core_node,
        "row_node": row_node,
    }


def build_program(chunks_u, cfg=Cfg, reps=1, do_gather="rr", do_onehot="t2",
                  do_mm=True):
    """chunks_u: [NB, NW] int array of 128-slot chunk counts per cell."""
    group_sizes = _group_sizes(cfg)
    tot_slots = int(chunks_u.sum()) * 128
    nc = bacc.Bacc("TRN2", target_bir_lowering=False, debug=True,
                   num_swdge_queues=cfg.NW)

    data = nc.dram_tensor("data", [cfg.N, cfg.DIN], BF16, kind="ExternalInput")
    idxs = nc.dram_tensor("idxs", [128, tot_slots // 16], I16,
                          kind="ExternalInput")
    locs = nc.dram_tensor("locs", [128, tot_slots // 128], BF16,
                          kind="ExternalInput")
    deg = nc.dram_tensor("deg", [1, cfg.NB * 128], BF16, kind="ExternalInput")
    iota_in = nc.dram_tensor("iota", [128, 128], BF16, kind="ExternalInput")
    cwmax = 0
    b0t = 0
    for gs in group_sizes:
        for w in range(cfg.NW):
            cwmax = max(cwmax, int(chunks_u[b0t:b0t + gs, w].sum()))
        b0t += gs
    if do_onehot == "t2":
        iotar_in = nc.dram_tensor("iotar", [128, 128 * cwmax], BF16,
                                  kind="ExternalInput")
    wcomb_in = nc.dram_tensor("wcomb", [cfg.DIN, cfg.DOUT], BF16,
                              kind="ExternalInput")
    bpw_in = nc.dram_tensor("bpw", [1, cfg.DOUT], BF16, kind="ExternalInput")
    bpost_in = nc.dram_tensor("bpost", [1, cfg.DOUT], BF16,
                              kind="ExternalInput")
    out = nc.dram_tensor("out", [cfg.DOUT, cfg.NB * 128], F32,
                         kind="ExternalOutput")

    with tile.TileContext(nc) as tc, ExitStack() as stk:
        nc.gpsimd.load_library(library_config.mlp)
        with (
            tc.tile_pool(name="consts", bufs=1) as cpool,
            tc.tile_pool(name="idxp", bufs=6) as idxp,
            tc.tile_pool(name="locp", bufs=6) as locp,
            tc.tile_pool(name="msgs", bufs=6) as msgsp,
            tc.tile_pool(name="oh", bufs=(3 if do_onehot == "group"
                                          else 5)) as ohp,
            tc.tile_pool(name="accsb", bufs=3) as accsbp,
            tc.tile_pool(name="outsb", bufs=2) as outsbp,
            tc.tile_pool(name="degp", bufs=2) as degp,
        ):
            iota_sb = cpool.tile([128, 128], BF16)
            if do_onehot == "t2":
                iotar_sb = cpool.tile([128, 128, cwmax], BF16)
                nc.sync.dma_start(out=iotar_sb[:], in_=iotar_in[:])
            wcomb_sb = cpool.tile([cfg.DIN, cfg.DOUT], BF16)
            bpw_sb = cpool.tile([1, cfg.DOUT], BF16)
            bpost_sb = cpool.tile([1, cfg.DOUT], BF16)
            ones_sb = cpool.tile([1, 128], BF16)
            nc.sync.dma_start(out=iota_sb[:], in_=iota_in[:])
            nc.sync.dma_start(out=wcomb_sb[:], in_=wcomb_in[:])
            nc.sync.dma_start(out=bpw_sb[:], in_=bpw_in[:])
            nc.sync.dma_start(out=bpost_sb[:], in_=bpost_in[:])
            nc.vector.memset(ones_sb[:], 1.0)
            m_shared = o_shared = None
            if not do_gather or not do_onehot:
                cmax = 0
                b0t = 0
                for gs in group_sizes:
                    for w in range(cfg.NW):
                        cw = int(chunks_u[b0t:b0t + gs, w].sum())
                        cmax = max(cmax, cw)
                    b0t += gs
                if not do_gather:
                    m_shared = cpool.tile([128, cmax, cfg.DIN], BF16)
                    nc.vector.memset(m_shared[:], 0.0)
                if not do_onehot:
                    o_shared = cpool.tile([128, cmax, 128], BF16)
                    nc.vector.memset(o_shared[:], 0.0)

            psacc = stk.enter_context(
                tc.tile_pool(name="psacc", bufs=cfg.G, space="PSUM"))
            psout = stk.enter_context(
                tc.tile_pool(name="psout", bufs=2, space="PSUM"))
            for _rep in range(reps):
                off = 0      # slot offset
                b0 = 0       # first block of group
                for gi, gs in enumerate(group_sizes):
                    deg_t = degp.tile([1, gs * 128], BF16)
                    nc.sync.dma_start(out=deg_t[:],
                                      in_=deg[:, b0 * 128: (b0 + gs) * 128])
                    accs = ([psacc.tile([128, 128], F32,
                                        name=f"acc{_rep}_{b0}_{i}",
                                        tag="acc")[:]
                             for i in range(gs)] if do_mm else [])
                    # per block: windows in which it has its first/last chunk
                    nch = [[int(chunks_u[b0 + bi, w]) for w in range(cfg.NW)]
                           for bi in range(gs)]
                    o_group = None
                    if do_onehot == "group":
                        Cg = int(chunks_u[b0:b0 + gs, :].sum())
                        locg_t = locp.tile([128, Cg], BF16)
                        nc.sync.dma_start(
                            out=locg_t[:],
                            in_=locs[:, off // 128: off // 128 + Cg])
                        o_group = ohp.tile([128, Cg, 128], BF16)
                        nc.vector.tensor_tensor(
                            out=o_group[:],
                            in0=locg_t[:].unsqueeze(2).broadcast_to(
                                [128, Cg, 128]),
                            in1=iota_sb[:].unsqueeze(1).broadcast_to(
                                [128, Cg, 128]),
                            op=mybir.AluOpType.is_equal)
                        cg0 = 0
                    for w in range(cfg.NW):
                        Cw = sum(nch[bi][w] for bi in range(gs))
                        n = Cw * 128
                        idx_t = idxp.tile([128, n // 16], I16)
                        nc.sync.dma_start(
                            out=idx_t[:],
                            in_=idxs[:, off // 16: (off + n) // 16])
                        if do_onehot is True or do_onehot == "t2":
                            loc_t = locp.tile([128, Cw], BF16)
                            nc.sync.dma_start(
                                out=loc_t[:],
                                in_=locs[:, off // 128: (off + n) // 128])
                        if do_gather == "rr":
                            m_t = msgsp.tile([128, Cw, cfg.DIN], BF16)
                            nc.gpsimd.dma_gather(
                                m_t[:], data[w * cfg.WS: (w + 1) * cfg.WS, :],
                                idx_t[:], n, n, cfg.DIN, single_packet=False,
                                queue_num=w)
                        elif do_gather:
                            m_t = msgsp.tile([128, Cw, cfg.DIN], BF16)
                            c0 = 0
                            for q in range(cfg.NW):
                                cq = Cw // cfg.NW + (1 if q < Cw % cfg.NW
                                                     else 0)
                                if cq == 0:
                                    continue
                                nq = cq * 128
                                nc.gpsimd.dma_gather(
                                    m_t[:, c0:c0 + cq, :],
                                    data[w * cfg.WS: (w + 1) * cfg.WS, :],
                                    idx_t[:, c0 * 8:(c0 + cq) * 8],
                                    nq, nq, cfg.DIN, single_packet=False,
                                    queue_num=q)
                                c0 += cq
                        else:
                            m_t = m_shared
                        if do_onehot == "group":
                            o_t = None
                        elif do_onehot == "t2":
                            # transposed one-hot [p, dst, chunk]: both inputs
                            # have packed innermost axes -> DVE 2x eligible
                            o_t = ohp.tile([128, 128, Cw], BF16, name="o_t")
                            nc.vector.tensor_tensor(
                                out=o_t[:],
                                in0=loc_t[:].unsqueeze(1).broadcast_to(
                                    [128, 128, Cw]),
                                in1=iotar_sb[:, :, :Cw],
                                op=mybir.AluOpType.is_equal)
                        elif do_onehot:
                            o_t = ohp.tile([128, Cw, 128], BF16)
                            nc.vector.tensor_tensor(
                                out=o_t[:],
                                in0=loc_t[:].unsqueeze(2).broadcast_to(
                                    [128, Cw, 128]),
                                in1=iota_sb[:].unsqueeze(1).broadcast_to(
                                    [128, Cw, 128]),
                                op=mybir.AluOpType.is_equal)
                        else:
                            o_t = o_shared
                        ch = 0
                        if do_mm:
                            for bi in range(gs):
                                for cu in range(nch[bi][w]):
                                    if do_onehot == "group":
                                        rhs_ap = o_group[:, cg0 + ch, :]
                                    elif do_onehot == "t2":
                                        rhs_ap = o_t[:, :, ch]
                                    else:
                                        rhs_ap = o_t[:, ch, :]
                                    nc.tensor.matmul(
                                        out=accs[bi],
                                        lhsT=m_t[:, ch, :],
                                        rhs=rhs_ap,
                                        start=(w == 0 and cu == 0),
                                        stop=(w == cfg.NW - 1
                                              and cu == nch[bi][w] - 1))
                                    ch += 1
                        if do_onehot == "group":
                            cg0 += Cw
                        off += n

                    out_t = outsbp.tile([cfg.DOUT, gs * 128], F32)
                    if do_mm:
                        for bi in range(gs):
                            acc_sb = accsbp.tile([128, 128], BF16)
                            nc.scalar.copy(acc_sb[:], accs[bi])
                            outp = psout.tile([cfg.DOUT, 128], F32)
                            nc.tensor.matmul(out=outp[:], lhsT=wcomb_sb[:],
                                             rhs=acc_sb[:], start=True,
                                             stop=False)
                            nc.tensor.matmul(out=outp[:], lhsT=bpw_sb[:],
                                             rhs=deg_t[:,
                                                       bi * 128:(bi + 1) * 128],
                                             start=False, stop=False)
                            nc.tensor.matmul(out=outp[:], lhsT=bpost_sb[:],
                                             rhs=ones_sb[:], start=False,
                                             stop=True)
                            nc.scalar.copy(out_t[:, bi * 128:(bi + 1) * 128],
                                           outp[:])
                    else:
                        nc.vector.memset(out_t[:], 0.0)
                    nc.sync.dma_start(
                        out=out[:, b0 * 128:(b0 + gs) * 128], in_=out_t[:])
                    b0 += gs
    nc.compile()
    return nc


_PROGRAM_CACHE = {}


def _get_program(chunks_u, cfg=Cfg, reps=1):
    key = (chunks_u.tobytes(), reps)
    if key not in _PROGRAM_CACHE:
        _PROGRAM_CACHE[key] = build_program(chunks_u, cfg, reps)
    return _PROGRAM_CACHE[key]


_PRE_CACHE = {}


def _preprocess_cached(edge_src, edge_dst, cfg=Cfg):
    import hashlib
    key = hashlib.sha1(np.asarray(edge_src).tobytes()
                       + np.asarray(edge_dst).tobytes()).hexdigest()
    if key not in _PRE_CACHE:
        _PRE_CACHE[key] = preprocess(edge_src, edge_dst, cfg)
    return _PRE_CACHE[key]


def make_in_maps(data, edge_src, edge_dst, W_pre, b_pre, W_post, b_post,
                 cfg=Cfg):
    pre = _preprocess_cached(edge_src, edge_dst, cfg)
    data16 = np.ascontiguousarray(
        np.asarray(data, dtype=np.float32)).astype(NPBF16)
    iota = np.tile(np.arange(128, dtype=np.float32),
                   (128, 1)).astype(NPBF16)
    wp = np.asarray(W_pre, np.float64)
    wq = np.asarray(W_post, np.float64)
    wcomb = (wp @ wq).astype(np.float32).astype(NPBF16)
    bpw = (np.asarray(b_pre, np.float64) @ wq).reshape(1, cfg.DOUT)
    bpw = bpw.astype(np.float32).astype(NPBF16)
    bpost = np.asarray(b_post, np.float32).reshape(1, cfg.DOUT).astype(NPBF16)
    group_sizes = _group_sizes(cfg)
    cwmax = 0
    b0t = 0
    for gs in group_sizes:
        for w in range(cfg.NW):
            cwmax = max(cwmax, int(pre["chunks_u"][b0t:b0t + gs, w].sum()))
        b0t += gs
    iotar = np.repeat(np.arange(128, dtype=np.float32), cwmax)
    iotar = np.tile(iotar, (128, 1)).astype(NPBF16)
    in_maps = []
    for c in range(cfg.NC):
        in_maps.append({
            "data": data16,
            "idxs": pre["idx_dram"][c],
            "locs": pre["loc_dram"][c],
            "deg": pre["deg_dram"][c],
            "iota": iota,
            "iotar": iotar,
            "wcomb": wcomb,
            "bpw": bpw,
            "bpost": bpost,
        })
    return pre, in_maps


def assemble_output(pre, core_outs, cfg=Cfg):
    """core_outs: list of [DOUT, NB*128] arrays -> full [N, DOUT]."""
    out = np.empty((cfg.N, cfg.DOUT), np.float32)
    core_node = pre["core_node"]
    row_node = pre["row_node"]
    for c in range(cfg.NC):
        mask = core_node == c
        out[mask] = core_outs[c][:, row_node[mask]].T
    return out


def kernel(data, edge_src, edge_dst, W_pre, b_pre, W_post, b_post):
    cfg = Cfg
    pre, in_maps = make_in_maps(data, edge_src, edge_dst, W_pre, b_pre,
                                W_post, b_post, cfg)
    nc = _get_program(pre["chunks_u"], cfg)
    try:
        res = run_bass_kernel_spmd(nc, in_maps, list(range(cfg.NC)),
                                   trace=False)
    except ModuleNotFoundError:
        # BASS_TRACE set but the axon NTFF hook isn't importable here —
        # retry with tracing force-disabled rather than failing the run.
        import os
        os.environ["BASS_NEVER_TRACE"] = "1"
        res = run_bass_kernel_spmd(nc, in_maps, list(range(cfg.NC)),
                                   trace=False)
    return assemble_output(pre, [res.results[c]["out"]
                                 for c in range(cfg.NC)], cfg)
